# revision 1
# baseline (speedup 1.0000x reference)
"""Corr2D cost-volume kernel for Trainium2 (Bass/Tile), 8-core SPMD.

Problem: in1, in2: [B=8, C=128, H=128, W=256] fp32.
Output: [B, 81, H, W] where out[b, dy*9+dx, h, w] =
    mean_c in1[b,c,h,w] * pad(in2)[b, c, h+dy, w+dx]   (pad=4 each side)

Sharding: data-parallel over batch; core b handles batch b. No collectives.

Per-core pipeline:
  - in2 resident in SBUF zero-padded [C, 136, 264] bf16; in1 streamed in
    16-row bands; both cast fp32->bf16 on-chip (loads via HWDGE).
  - Patch = one output row x 128 w-pixels (stationary must be a single
    strided free dim on the PE).  lhsT = in1[c, h, wb:wb+128]; rhs =
    in2p[c, h:h+9, wb:wb+136] streamed as N = 9*136 = 1224 columns in
    3 matmuls of N=408: psum[p, yq*136+xq] = sum_c in1*in2p.
    Useful entries sit at n = 136*dy + p + dx -- a per-partition (p)
    offset that no on-chip engine or DMA can apply (HWDGE wraps byte
    shifts every 4 partitions; engines are partition-uniform).
  - De-shear via DRAM, where addressing is flat: band S (bf16) -> DRAM G
    (32 patches per G); per 16-row band, 9 gather-DMAs (one per dy) with
    src AP [[32*1224+1, 128], [1224, 32], [1, 9]] + offset 136*dy read
    back CT[p, patch, dy*9+dx] -- partition-uniform.
  - PE transpose CT-slice [128, 81] -> [81, 128] psum (bf16 identity),
    ACT copies scaled by 1/128 into staging Z[81, 16, 256]; one DMA per
    band stores Z with 1KB-contiguous runs into out[81, H, W].
"""

import os
import sys

for _p in ("/opt/trn_rl_repo",):
    if _p not in sys.path and os.path.isdir(_p):
        sys.path.insert(0, _p)

import numpy as np

import concourse.bass as bass
import concourse.tile as tile
from concourse import bacc, mybir
from concourse.bass_utils import run_bass_kernel_spmd

# Problem constants (hardcoded per harness contract)
B, C, H, W = 8, 128, 128, 256
MD = 4
K9 = 2 * MD + 1          # 9
K81 = K9 * K9            # 81
HP, WP = H + 2 * MD, W + 2 * MD   # 136, 264
DW = 128 + 2 * MD        # 136 window width per 128-px chunk
NBAND = K9 * DW          # 1224 columns per patch
NCHUNK = 2               # w chunks per row
ZH = 16                  # rows per band
NP = ZH * NCHUNK         # 32 patches per band

F32 = mybir.dt.float32
BF16 = mybir.dt.bfloat16


def build_nc(nbands=None, nchunks=None):
    nbands = H // ZH if nbands is None else nbands
    nchunks = NCHUNK if nchunks is None else nchunks
    nc = bacc.Bacc(None, target_bir_lowering=False)

    in1_d = nc.dram_tensor("in1", (C, H, W), F32, kind="ExternalInput")
    in2_d = nc.dram_tensor("in2", (C, H, W), F32, kind="ExternalInput")
    ident_d = nc.dram_tensor("ident", (128, 128), BF16, kind="ExternalInput")
    out_d = nc.dram_tensor("out", (K81, H, W), F32, kind="ExternalOutput")

    with tile.TileContext(nc) as tc:
        with (
            tc.tile_pool(name="const", bufs=1) as constp,
            tc.tile_pool(name="ld", bufs=2) as ldp,
            tc.tile_pool(name="in1p", bufs=2) as in1p,
            tc.tile_pool(name="sp", bufs=4) as sp,
            tc.tile_pool(name="cp", bufs=2) as cp,
            tc.tile_pool(name="zp", bufs=2) as zp,
            tc.tile_pool(name="mmps", bufs=6, space="PSUM") as mmps,
            tc.tile_pool(name="tps", bufs=2, space="PSUM") as tpsp,
            tc.tile_pool(name="gp", bufs=2, space="DRAM") as gp,
        ):
            ident = constp.tile([128, 128], BF16)
            nc.sync.dma_start(out=ident[:], in_=ident_d[:])

            # resident zero-padded bf16 in2: [C, HP, WP]
            in2p = constp.tile([C, HP, WP], BF16)
            nc.vector.memset(in2p[:], 0)
            CH = 8  # rows per load chunk
            for ci in range(H // CH):
                tmp2 = ldp.tile([C, CH, W], F32, tag="ldtmp")
                nc.sync.dma_start(
                    out=tmp2[:], in_=in2_d[:, ci * CH : (ci + 1) * CH, :]
                )
                dst = in2p[:, MD + ci * CH : MD + (ci + 1) * CH, MD : MD + W]
                if ci % 2 == 0:
                    nc.vector.tensor_copy(dst, tmp2[:])
                else:
                    nc.scalar.copy(dst, tmp2[:])

            dumps_hist = []  # per band: list of dump BassInstructions
            gath_hist = []   # per band: list of gather BassInstructions
            trans_hist = []  # per band: list of transpose BassInstructions
            for band in range(nbands):
                hb = band * ZH
                # load+cast one 16-row band of in1
                tmp1a = ldp.tile([C, CH, W], F32, tag="ldtmp")
                nc.sync.dma_start(out=tmp1a[:], in_=in1_d[:, hb : hb + CH, :])
                tmp1b = ldp.tile([C, CH, W], F32, tag="ldtmp")
                nc.sync.dma_start(
                    out=tmp1b[:], in_=in1_d[:, hb + CH : hb + ZH, :]
                )
                in1b = in1p.tile([C, ZH, W], BF16)
                nc.vector.tensor_copy(in1b[:, 0:CH, :], tmp1a[:])
                nc.scalar.copy(in1b[:, CH:ZH, :], tmp1b[:])

                zt = zp.tile([K81, ZH, W], F32)
                gt = gp.tile([128, NP * NBAND], BF16)
                grow = gt.ap[0][0]
                dumps, gaths, trans = [], [], []

                # compute + dump all 32 patches of this band
                for hi in range(ZH):
                    h = hb + hi
                    for wc in range(nchunks):
                        wb = wc * 128
                        pi = hi * NCHUNK + wc
                        lhsT = in1b[:, hi, wb : wb + 128]
                        st = sp.tile([128, NBAND], BF16)
                        for j in range(3):
                            rhs = in2p[
                                :, h + 3 * j : h + 3 * j + 3, wb : wb + DW
                            ]
                            ps = mmps.tile([128, 3 * DW], F32)
                            nc.tensor.matmul(
                                ps[:], lhsT, rhs, start=True, stop=True
                            )
                            dstS = st[:, 3 * DW * j : 3 * DW * (j + 1)]
                            if (pi + j) % 2 == 0:
                                nc.vector.tensor_copy(dstS, ps[:])
                            else:
                                nc.scalar.copy(dstS, ps[:])
                        # dump band to DRAM bounce
                        gdst = bass.AP(
                            gt.tensor,
                            pi * NBAND,
                            [[grow, 128], [1, NBAND]],
                        )
                        dumps.append(nc.sync.dma_start(out=gdst, in_=st[:]))

                # de-shear gather: 9 DMAs for the whole band
                ct = cp.tile([128, NP * K81], BF16)
                crow = ct.ap[0][0]
                for dy in range(K9):
                    src = bass.AP(
                        gt.tensor,
                        dy * DW,
                        [[grow + 1, 128], [NBAND, NP], [1, K9]],
                    )
                    cdst = bass.AP(
                        ct.tensor,
                        dy * K9,
                        [[crow, 128], [K81, NP], [1, K9]],
                    )
                    gaths.append(nc.sync.dma_start(out=cdst, in_=src))

                # transpose + scaled store per patch
                for hi in range(ZH):
                    for wc in range(nchunks):
                        wb = wc * 128
                        pi = hi * NCHUNK + wc
                        tt = tpsp.tile([K81, 128], BF16)
                        trans.append(
                            nc.tensor.transpose(
                                tt[:],
                                ct[:, pi * K81 : (pi + 1) * K81],
                                ident[:],
                            )
                        )
                        nc.scalar.mul(
                            zt[:, hi, wb : wb + 128], tt[:], 1.0 / C
                        )

                nc.sync.dma_start(out=out_d[:, hb : hb + ZH, :], in_=zt[:])

                # Explicit dependency edges for raw-AP DMA accesses the
                # Tile tracker cannot range-analyze:
                #   RAW: gathers read G after all dumps; transposes read
                #        CT after all gathers.
                #   WAR: G/CT pool slots (bufs=2) must not be reused
                #        until the prior band's readers finished.
                for g in gaths:
                    for d in dumps:
                        bass._add_dep_helper(
                            g.ins, d.ins, sync=True, reason="gather RAW dump"
                        )
                for t in trans:
                    for g in gaths:
                        bass._add_dep_helper(
                            t.ins, g.ins, sync=True, reason="transp RAW gather"
                        )
                if len(dumps_hist) >= 2:
                    for d in dumps:
                        for g in gath_hist[-2]:
                            bass._add_dep_helper(
                                d.ins, g.ins, sync=True, reason="G WAR"
                            )
                    for g in gaths:
                        for t in trans_hist[-2]:
                            bass._add_dep_helper(
                                g.ins, t.ins, sync=True, reason="CT WAR"
                            )
                dumps_hist.append(dumps)
                gath_hist.append(gaths)
                trans_hist.append(trans)

    return nc


_NC = None


def _get_nc():
    global _NC
    if _NC is None:
        nc = build_nc()
        nc.compile()
        _NC = nc
    return _NC


def kernel(in1, in2):
    import ml_dtypes

    in1 = np.asarray(in1, dtype=np.float32)
    in2 = np.asarray(in2, dtype=np.float32)
    nc = _get_nc()
    ident = np.eye(128, dtype=ml_dtypes.bfloat16)
    in_maps = [
        {
            "in1": np.ascontiguousarray(in1[b]),
            "in2": np.ascontiguousarray(in2[b]),
            "ident": ident,
        }
        for b in range(B)
    ]
    res = run_bass_kernel_spmd(nc, in_maps, list(range(B)))
    return np.stack([r["out"] for r in res.results], axis=0)

